# revision 25
# baseline (speedup 1.0000x reference)
"""Trainium2 Bass kernel for nn_MultiHeadAttention (B=4, S=2048, C=256, H=8).

Sharding: data-parallel over (batch, seq) - 8 cores, core i handles
batch b = i//2 and query rows r0 = (i%2)*1024.  No collectives; the host
concatenates the 8 row-shards.

Algebraic restructuring (host-folded weights):
  scores = x @ (Wq Wk^T) @ x^T  -> one "w" projection replaces Q and K
           (bk's per-query score offsets are softmax-invariant; bq's
           per-key offsets enter through the exp bias, which is a
           per-partition AP on the score tiles = per-key).
  head_out @ Wfc = (attn @ x) @ (Wv Wfc) -> V projection eliminated;
           attn@V consumes x directly, fc uses G = 32*Wv@Wfc per head.

All matmuls are fp8e4 DoubleRow: 256-deep contraction streaming 2 rhs
columns/cycle (2x bf16 throughput).  x is host-transposed (no PE
transposes).  Score tiles are [key=128, q=1024] two-bank PSUM tiles; exp
runs on ACT only (one op per key tile; GPSIMD is too slow for bulk work
and cannot read PSUM).  attn@V runs in two q-half passes so PSUM fits:
st 2x2 + ot 2 + rs 2 = 8 banks.  fc accumulates all 8 heads in PSUM via
DoubleRow from the persistent ot8 buffer; its drain fuses the 1/512
descale with the residual (bfc + bv@Wfc host-folded into x rows).
LayerNorm runs in fp32 at the tail (single ACT sqrt-table load, after
all exps).

Scales: ones-matrix = 1/16 so reciprocal gives 16/rowsum (ot8 ~ x16 in
good fp8 range); m8 = 16*(Wq Wk^T)^T with w-drain x1/16; g8 = 32*Wv@Wfc;
fc drain x1/512.  exp computes exp(s/16 - 2): softmax-invariant bias
keeping e under ~45 (fp8e4 max 240).
"""

import sys

for _p in ("/opt/trn_rl_repo",):
    if _p not in sys.path:
        sys.path.insert(0, _p)

from contextlib import ExitStack

import numpy as np

import concourse.bass as bass
from concourse import bacc
import concourse.tile as tile
from concourse import mybir

P = 128
B, S, C, H = 4, 2048, 256, 8
RQ = 1024            # query rows per core
NT = S // P          # key tiles = 16
NP2 = NT // 2        # key tile pairs = 8
ND = C // P          # d tiles = 2
NR = RQ // P         # row tiles per core = 8
QH = RQ // 2         # q half = 512
EPS = 1e-5
SCALE = 1.0 / np.sqrt(C)    # 1/16
EXP_BIAS = -2.0             # exp(s*SCALE + EXP_BIAS); softmax-invariant

F32 = mybir.dt.float32
I32 = mybir.dt.int32
FP8 = mybir.dt.float8e4
AF = mybir.ActivationFunctionType
OP = mybir.AluOpType
DR = mybir.MatmulPerfMode.DoubleRow

# Schraudolph fast exp: exp(x) ~ bitcast_f32(int32(A*x + B)); key tiles in
# TRICK_T run it on DVE (step1 int32 + step2 fp8 convert) to relieve ACT
A_EXP = float(2**23 / np.log(2.0))
B_EXP = float(127 * 2**23 - 366000.0)
TRICK_T = {2, 7, 12, 15}


def build_nc(plain_affine: bool = False) -> bass.Bass:
    nc = bacc.Bacc(None)

    xt8 = nc.declare_dram_parameter("xt8", [P, ND, S], FP8, isOutput=False)
    x8r = nc.declare_dram_parameter("x8r", [P, NT, C], FP8, isOutput=False)
    xqf = nc.declare_dram_parameter("xqf", [RQ, C], F32, isOutput=False)
    m8 = nc.declare_dram_parameter("m8", [P, ND, H, C], FP8, isOutput=False)
    g8 = nc.declare_dram_parameter("g8", [P, ND, H, C], FP8, isOutput=False)
    grow = nc.declare_dram_parameter("grow", [2 * C], F32, isOutput=False)
    # per-key exp bias EXP_BIAS + SCALE*beta[h, key] (constant when bq=0)
    eba = nc.declare_dram_parameter("eba", [P, H, NT], F32, isOutput=False)
    out = nc.declare_dram_parameter("out", [RQ, C], F32, isOutput=True)

    with tile.TileContext(nc) as tc, ExitStack() as ctx:
        singles = ctx.enter_context(tc.tile_pool(name="singles", bufs=1))
        wpool = ctx.enter_context(tc.tile_pool(name="wpool", bufs=2))
        epool = ctx.enter_context(tc.tile_pool(name="epool", bufs=2))
        tpool = ctx.enter_context(tc.tile_pool(name="tpool", bufs=2))
        rpool = ctx.enter_context(tc.tile_pool(name="rpool", bufs=2))
        lnpool = ctx.enter_context(tc.tile_pool(name="lnpool", bufs=8))

        pp = ctx.enter_context(tc.tile_pool(name="pp", bufs=2, space="PSUM"))
        pot = ctx.enter_context(tc.tile_pool(name="pot", bufs=1, space="PSUM"))
        prs = ctx.enter_context(tc.tile_pool(name="prs", bufs=1, space="PSUM"))

        # ---- constants ----
        ones8 = singles.tile([P, ND, P], FP8)
        nc.vector.memset(ones8, float(SCALE))        # 1/16: rcp -> 16/rowsum
        eps_t = singles.tile([P, 1], F32)
        nc.vector.memset(eps_t, EPS)

        # ---- input DMAs (parallel queues: SP / ACT / Pool DGEs) ----
        xt_sb = singles.tile([P, ND, S], FP8)
        nc.sync.dma_start(out=xt_sb, in_=xt8[:])
        m_sb = singles.tile([P, ND, H, C], FP8)
        nc.scalar.dma_start(out=m_sb, in_=m8[:])
        xr8_sb = singles.tile([P, NT, C], FP8)
        nc.gpsimd.dma_start(out=xr8_sb, in_=x8r[:])
        g_sb = singles.tile([P, ND, H, C], FP8)
        nc.scalar.dma_start(out=g_sb, in_=g8[:])
        eba_sb = singles.tile([P, H, NT], F32)
        nc.sync.dma_start(out=eba_sb, in_=eba[:])
        # Schraudolph intercepts: B_EXP + A_EXP * eba (per key)
        ebt_sb = singles.tile([P, H, NT], F32)
        nc.vector.tensor_scalar(out=ebt_sb, in0=eba_sb[:], scalar1=A_EXP,
                                scalar2=B_EXP, op0=OP.mult, op1=OP.add)
        xq_sb = singles.tile([P, NR, C], F32)
        nc.gpsimd.dma_start(out=xq_sb, in_=xqf.rearrange("(n p) d -> p n d", p=P))
        grow_sb = singles.tile([P, 2 * C], F32)
        grow_ap = grow[:]
        grow_bc = bass.AP(tensor=grow_ap.tensor, offset=grow_ap.offset,
                          ap=[[0, P]] + list(grow_ap.ap))
        nc.sync.dma_start(out=grow_sb, in_=grow_bc)
        gamma_sb = grow_sb[:, 0:C]
        beta_sb = grow_sb[:, C:2 * C]

        # ---- persistent buffers ----
        ot8 = singles.tile([P, ND, H, RQ], FP8)      # all-heads attn-out (x16)
        ln_out = singles.tile([P, NR, C], F32)
        out_r = out.rearrange("(n p) d -> p n d", p=P)

        # ---- PE warmup: ramp the clock while input DMAs land ----
        wps = pp.tile([P, 2 * QH], F32, tag="pp", name="wps")
        for _ in range(24):
            nc.tensor.matmul(wps[:, 0:P], lhsT=ones8[:], rhs=ones8[:],
                             start=True, stop=True, perf_mode=DR)

        # w projection part for one head -> w8 [p(d_lo), d2, t] fp8 (x1/16);
        # part idx in 0..3 covers (d2, t2) - emitted spread across pass 0
        def emit_wproj_part(h, w8, idx):
            d2, t2 = idx // 2, idx % 2
            ps = pp.tile([P, 2 * QH], F32, tag="pp")
            for j in range(2):
                nc.tensor.matmul(
                    ps[:, j * QH:(j + 1) * QH],
                    lhsT=m_sb[:, :, h, d2 * P:(d2 + 1) * P],
                    rhs=xt_sb[:, :, (2 * t2 + j) * QH:(2 * t2 + j + 1) * QH],
                    start=True, stop=True, perf_mode=DR,
                )
            nc.vector.tensor_scalar_mul(
                out=w8[:, d2, 2 * t2 * QH:(2 * t2 + 2) * QH], in0=ps,
                scalar1=1.0 / 16.0)

        w8 = wpool.tile([P, ND, S], FP8, tag="w")
        for idx in range(4):
            emit_wproj_part(0, w8, idx)

        # fc (PSUM-accumulated over all heads) + residual + LN for one
        # 128-row tile; emitted inside the last head so it overlaps
        def emit_fc_ln(rt):
            psf = pp.tile([P, 2 * QH], F32, tag="pp", name="psf")
            for h2 in range(H):
                nc.tensor.matmul(
                    psf[:, 0:C],
                    lhsT=ot8[:, :, h2, rt * P:(rt + 1) * P],
                    rhs=g_sb[:, :, h2, :],
                    start=(h2 == 0), stop=(h2 == H - 1), perf_mode=DR,
                )
            t = lnpool.tile([P, C], F32, tag="lnt")
            nc.vector.scalar_tensor_tensor(
                out=t, in0=psf[:, 0:C], scalar=1.0 / 512.0, in1=xq_sb[:, rt],
                op0=OP.mult, op1=OP.add)
            stats = lnpool.tile([P, 6], F32, tag="stats")
            nc.vector.bn_stats(out=stats, in_=t)
            mv = lnpool.tile([P, 2], F32, tag="mv")
            nc.vector.bn_aggr(out=mv, in_=stats)
            sd = lnpool.tile([P, 1], F32, tag="sd")
            nc.scalar.activation(out=sd, in_=mv[:, 1:2], func=AF.Sqrt,
                                 bias=eps_t, scale=1.0)
            rstd = lnpool.tile([P, 1], F32, tag="rstd")
            nc.vector.reciprocal(out=rstd, in_=sd)
            if plain_affine:  # gamma == 1, beta == 0
                nc.vector.tensor_scalar(
                    out=ln_out[:, rt], in0=t, scalar1=mv[:, 0:1],
                    scalar2=rstd, op0=OP.subtract, op1=OP.mult)
            else:
                nc.vector.tensor_scalar(out=t, in0=t, scalar1=mv[:, 0:1],
                                        scalar2=rstd, op0=OP.subtract,
                                        op1=OP.mult)
                nc.vector.tensor_tensor(out=t, in0=t, in1=gamma_sb, op=OP.mult)
                nc.vector.tensor_tensor(out=ln_out[:, rt], in0=t, in1=beta_sb,
                                        op=OP.add)
            nc.sync.dma_start(out=out_r[:, rt:rt + 1, :],
                              in_=ln_out[:, rt:rt + 1])

        # ---- head loop ----
        for h in range(H):
            e8 = epool.tile([P, NT, RQ], FP8, tag="e")
            rs_ps = prs.tile([P, RQ], F32, tag="rs")
            st_tiles = {}

            def emit_score(t):
                st = pp.tile([P, RQ], F32, tag="pp", name="st")
                for j in range(2):
                    nc.tensor.matmul(
                        st[:, j * QH:(j + 1) * QH],
                        lhsT=w8[:, :, t * P:(t + 1) * P],
                        rhs=xt_sb[:, :, j * QH:(j + 1) * QH],
                        start=True, stop=True, perf_mode=DR,
                    )
                st_tiles[t] = st

            def emit_exp(t):
                st = st_tiles.pop(t)
                if t in TRICK_T:
                    ti = tpool.tile([P, RQ], I32, tag="ti")
                    nc.vector.tensor_scalar(
                        out=ti, in0=st, scalar1=float(SCALE * A_EXP),
                        scalar2=ebt_sb[:, h, t:t + 1],
                        op0=OP.mult, op1=OP.add)
                    nc.vector.tensor_copy(out=e8[:, t], in_=ti[:].bitcast(F32))
                else:
                    nc.scalar.activation(out=e8[:, t], in_=st, func=AF.Exp,
                                         bias=eba_sb[:, h, t:t + 1],
                                         scale=float(SCALE))

            # pass 0: scores (pipelined two ahead) + exp + rowsum + attnV qh0
            ot_a = [pot.tile([P, QH], F32, tag="ot", name=f"ota{d2}")
                    for d2 in range(ND)]
            emit_score(0)
            emit_exp(0)
            emit_score(1)
            emit_exp(1)
            w8_next = (wpool.tile([P, ND, S], FP8, tag="w", name="w8n")
                       if h + 1 < H else None)
            for tp in range(NP2):
                if tp + 1 < NP2:
                    emit_score(2 * tp + 2)
                    emit_exp(2 * tp + 2)
                    emit_score(2 * tp + 3)
                    emit_exp(2 * tp + 3)
                pair = slice(2 * tp, 2 * tp + 2)
                for j in range(2):
                    nc.tensor.matmul(rs_ps[:, j * QH:(j + 1) * QH],
                                     lhsT=ones8[:],
                                     rhs=e8[:, pair, j * QH:(j + 1) * QH],
                                     start=(tp == 0), stop=(tp == NP2 - 1),
                                     perf_mode=DR)
                for d2 in range(ND):
                    nc.tensor.matmul(
                        ot_a[d2],
                        lhsT=xr8_sb[:, pair, d2 * P:(d2 + 1) * P],
                        rhs=e8[:, pair, 0:QH],
                        start=(tp == 0), stop=(tp == NP2 - 1),
                        perf_mode=DR,
                    )
                # next head's w projection rides along, one part per pair
                if w8_next is not None and 2 <= tp <= 5:
                    emit_wproj_part(h + 1, w8_next, tp - 2)

            rcp_f = rpool.tile([P, RQ], F32, tag="rcp")
            nc.vector.reciprocal_approx_fast(out=rcp_f, in_=rs_ps)
            for d2 in range(ND):
                nc.vector.tensor_tensor(
                    out=ot8[:, d2, h, 0:QH], in0=ot_a[d2],
                    in1=rcp_f[:, 0:QH], op=OP.mult)

            # on the last head, fc+LN of the qh0 row tiles overlaps qh1
            if h == H - 1:
                for rt in range(NR // 2):
                    emit_fc_ln(rt)

            # pass 1: attnV qh1
            ot_b = [pot.tile([P, QH], F32, tag="ot", name=f"otb{d2}")
                    for d2 in range(ND)]
            for tp in range(NP2):
                pair = slice(2 * tp, 2 * tp + 2)
                for d2 in range(ND):
                    nc.tensor.matmul(
                        ot_b[d2],
                        lhsT=xr8_sb[:, pair, d2 * P:(d2 + 1) * P],
                        rhs=e8[:, pair, QH:RQ],
                        start=(tp == 0), stop=(tp == NP2 - 1),
                        perf_mode=DR,
                    )
            if h + 1 < H:
                for d2 in range(ND):
                    nc.vector.tensor_tensor(
                        out=ot8[:, d2, h, QH:RQ], in0=ot_b[d2],
                        in1=rcp_f[:, QH:RQ], op=OP.mult)
                w8 = w8_next
            else:
                # last head: normalize + fc + LN per row tile for a tight tail
                for rt in range(NR // 2, NR):
                    qsl = slice(rt * P, (rt + 1) * P)
                    bsl = slice((rt - NR // 2) * P, (rt - NR // 2 + 1) * P)
                    for d2 in range(ND):
                        nc.vector.tensor_tensor(
                            out=ot8[:, d2, h, qsl], in0=ot_b[d2][:, bsl],
                            in1=rcp_f[:, qsl], op=OP.mult)
                    emit_fc_ln(rt)

    nc.finalize()
    return nc


_NC = {}


def _get_nc(plain_affine: bool = False):
    # score bias rides the eba input tensor (no graph change); the trivial
    # gamma/beta fast path is a build-time flag
    if plain_affine not in _NC:
        _NC[plain_affine] = build_nc(plain_affine)
    return _NC[plain_affine]


def _to_fp8(a):
    import ml_dtypes
    return np.clip(np.asarray(a, np.float32), -240.0, 240.0).astype(
        ml_dtypes.float8_e4m3)


def _wproj8(w, scale):  # [H, C, C] -> [P, ND, H, C] fp8
    w = np.asarray(w, np.float32) * scale
    return np.ascontiguousarray(_to_fp8(
        w.reshape(H, ND, P, C).transpose(2, 1, 0, 3)))


def make_in_maps(inputs):
    x = np.asarray(inputs["x"], dtype=np.float32)
    Wq = np.asarray(inputs["Wq"], np.float32)
    Wk = np.asarray(inputs["Wk"], np.float32)
    Wv = np.asarray(inputs["Wv"], np.float32)
    wfc_f = np.asarray(inputs["Wfc"], np.float32)
    bq = np.asarray(inputs["bq"], np.float32)
    bk = np.asarray(inputs["bk"], np.float32)
    bfc_eff = (np.asarray(inputs["bfc"], np.float32).ravel()
               + np.asarray(inputs["bv"], np.float32).ravel() @ wfc_f)

    # scores = x @ M @ x^T with M = Wq @ Wk^T ; lhsT block needs M^T = Wk@Wq^T
    mT = np.einsum("hcx,hdx->hcd", Wk, Wq)           # [H, C, C] = (WqWk^T)^T
    gfc = np.einsum("hcd,hde->hce", Wv, wfc_f.reshape(H, C, C))
    wkbq = np.einsum("hcd,hd->hc", Wk, bq)           # [H, C]
    biased = bool(np.abs(bq).max() > 0)
    gamma = np.asarray(inputs["gamma"], np.float32).ravel()
    beta = np.asarray(inputs["beta"], np.float32).ravel()
    plain_affine = bool((gamma == 1.0).all() and (beta == 0.0).all())

    shared = {
        "m8": _wproj8(mT, 16.0),
        "g8": _wproj8(gfc, 32.0),
        "grow": np.ascontiguousarray(np.concatenate([
            np.asarray(inputs["gamma"], np.float32).ravel(),
            np.asarray(inputs["beta"], np.float32).ravel(),
        ])),
    }
    x8 = _to_fp8(x)
    in_maps = []
    for core in range(8):
        b, r0 = core // 2, (core % 2) * RQ
        m = dict(shared)
        xroll = np.roll(x8[b], -r0, axis=0)          # own query rows first
        m["xt8"] = np.ascontiguousarray(
            xroll.T.reshape(ND, P, S).transpose(1, 0, 2))
        m["x8r"] = np.ascontiguousarray(
            xroll.reshape(NT, P, C).transpose(1, 0, 2))
        m["xqf"] = np.ascontiguousarray(x[b, r0:r0 + RQ] + bfc_eff[None, :])
        if biased:
            xrf = np.roll(x[b], -r0, axis=0)
            beta_sc = xrf @ wkbq.T + (bq * bk).sum(-1)[None, :]   # [S, H]
            ebv = (EXP_BIAS + SCALE * beta_sc).T.reshape(H, NT, P)
            m["eba"] = np.ascontiguousarray(
                ebv.transpose(2, 0, 1).astype(np.float32))
        else:
            m["eba"] = np.full((P, H, NT), EXP_BIAS, np.float32)
        in_maps.append(m)
    return in_maps, plain_affine


def assemble(results):
    out = np.empty((B, S, C), dtype=np.float32)
    for core in range(8):
        b, r0 = core // 2, (core % 2) * RQ
        out[b, r0:r0 + RQ] = results[core]["out"]
    return out


def kernel(**inputs) -> np.ndarray:
    from concourse.bass_utils import run_bass_kernel_spmd

    in_maps, plain_affine = make_in_maps(inputs)
    nc = _get_nc(plain_affine)
    res = run_bass_kernel_spmd(nc, in_maps, core_ids=list(range(8)))
    return assemble(res.results)


# revision 26
# speedup vs baseline: 1.1945x; 1.1945x over previous
"""Trainium2 Bass kernel for nn_MultiHeadAttention (B=4, S=2048, C=256, H=8).

Sharding: data-parallel over (batch, seq) - 8 cores, core i handles
batch b = i//2 and query rows r0 = (i%2)*1024.  No collectives; the host
concatenates the 8 row-shards.

Algebraic restructuring (host-folded weights):
  scores = x @ (Wq Wk^T) @ x^T  -> one "w" projection replaces Q and K
           (bk's per-query score offsets are softmax-invariant; bq's
           per-key offsets enter through the exp bias, which is a
           per-partition AP on the score tiles = per-key).
  head_out @ Wfc = (attn @ x) @ (Wv Wfc) -> V projection eliminated;
           attn@V consumes x directly, fc uses G = 32*Wv@Wfc per head.

All matmuls are fp8e4 DoubleRow: 256-deep contraction streaming 2 rhs
columns/cycle (2x bf16 throughput).  x is host-transposed (no PE
transposes).  Score tiles are [key=128, q=1024] two-bank PSUM tiles; exp
runs on ACT only (one op per key tile; GPSIMD is too slow for bulk work
and cannot read PSUM).  attn@V runs in two q-half passes so PSUM fits:
st 2x2 + ot 2 + rs 2 = 8 banks.  fc accumulates all 8 heads in PSUM via
DoubleRow from the persistent ot8 buffer; its drain fuses the 1/512
descale with the residual (bfc + bv@Wfc host-folded into x rows).
LayerNorm runs in fp32 at the tail (single ACT sqrt-table load, after
all exps).

Scales: ones-matrix = 1/16 so reciprocal gives 16/rowsum (ot8 ~ x16 in
good fp8 range); m8 = 16*(Wq Wk^T)^T with w-drain x1/16; g8 = 32*Wv@Wfc;
fc drain x1/512.  exp computes exp(s/16 - 2): softmax-invariant bias
keeping e under ~45 (fp8e4 max 240).
"""

import sys

for _p in ("/opt/trn_rl_repo",):
    if _p not in sys.path:
        sys.path.insert(0, _p)

from contextlib import ExitStack

import numpy as np

import concourse.bass as bass
from concourse import bacc
import concourse.tile as tile
from concourse import mybir

P = 128
B, S, C, H = 4, 2048, 256, 8
RQ = 1024            # query rows per core
NT = S // P          # key tiles = 16
NP2 = NT // 2        # key tile pairs = 8
ND = C // P          # d tiles = 2
NR = RQ // P         # row tiles per core = 8
QH = RQ // 2         # q half = 512
EPS = 1e-5
SCALE = 1.0 / np.sqrt(C)    # 1/16
EXP_BIAS = -2.0             # exp(s*SCALE + EXP_BIAS); softmax-invariant

F32 = mybir.dt.float32
I32 = mybir.dt.int32
FP8 = mybir.dt.float8e4
AF = mybir.ActivationFunctionType
OP = mybir.AluOpType
DR = mybir.MatmulPerfMode.DoubleRow

# Schraudolph fast exp: exp(x) ~ bitcast_f32(int32(A*x + B)); key tiles in
# TRICK_T run it on DVE (step1 int32 + step2 fp8 convert) to relieve ACT
A_EXP = float(2**23 / np.log(2.0))
B_EXP = float(127 * 2**23 - 366000.0)
TRICK_T = {5, 10, 15}


def build_nc(plain_affine: bool = False) -> bass.Bass:
    nc = bacc.Bacc(None)

    xt8 = nc.declare_dram_parameter("xt8", [P, ND, S], FP8, isOutput=False)
    x8r = nc.declare_dram_parameter("x8r", [P, NT, C], FP8, isOutput=False)
    xqf = nc.declare_dram_parameter("xqf", [RQ, C], F32, isOutput=False)
    m8 = nc.declare_dram_parameter("m8", [P, ND, H, C], FP8, isOutput=False)
    g8 = nc.declare_dram_parameter("g8", [P, ND, H, C], FP8, isOutput=False)
    grow = nc.declare_dram_parameter("grow", [2 * C], F32, isOutput=False)
    # per-key exp bias EXP_BIAS + SCALE*beta[h, key] (constant when bq=0)
    eba = nc.declare_dram_parameter("eba", [P, H, NT], F32, isOutput=False)
    out = nc.declare_dram_parameter("out", [RQ, C], F32, isOutput=True)

    with tile.TileContext(nc) as tc, ExitStack() as ctx:
        singles = ctx.enter_context(tc.tile_pool(name="singles", bufs=1))
        wpool = ctx.enter_context(tc.tile_pool(name="wpool", bufs=2))
        epool = ctx.enter_context(tc.tile_pool(name="epool", bufs=2))
        tpool = ctx.enter_context(tc.tile_pool(name="tpool", bufs=2))
        rpool = ctx.enter_context(tc.tile_pool(name="rpool", bufs=2))
        lnpool = ctx.enter_context(tc.tile_pool(name="lnpool", bufs=8))

        pp = ctx.enter_context(tc.tile_pool(name="pp", bufs=2, space="PSUM"))
        pot = ctx.enter_context(tc.tile_pool(name="pot", bufs=1, space="PSUM"))
        prs = ctx.enter_context(tc.tile_pool(name="prs", bufs=1, space="PSUM"))

        # ---- constants ----
        ones8 = singles.tile([P, ND, P], FP8)
        nc.vector.memset(ones8, float(SCALE))        # 1/16: rcp -> 16/rowsum
        eps_t = singles.tile([P, 1], F32)
        nc.vector.memset(eps_t, EPS)

        # ---- input DMAs (parallel queues: SP / ACT / Pool DGEs) ----
        xt_sb = singles.tile([P, ND, S], FP8)
        nc.sync.dma_start(out=xt_sb, in_=xt8[:])
        m_sb = singles.tile([P, ND, H, C], FP8)
        nc.scalar.dma_start(out=m_sb, in_=m8[:])
        xr8_sb = singles.tile([P, NT, C], FP8)
        nc.gpsimd.dma_start(out=xr8_sb, in_=x8r[:])
        g_sb = singles.tile([P, ND, H, C], FP8)
        nc.scalar.dma_start(out=g_sb, in_=g8[:])
        eba_sb = singles.tile([P, H, NT], F32)
        nc.sync.dma_start(out=eba_sb, in_=eba[:])
        # Schraudolph intercepts: B_EXP + A_EXP * eba (per key)
        ebt_sb = singles.tile([P, H, NT], F32)
        nc.vector.tensor_scalar(out=ebt_sb, in0=eba_sb[:], scalar1=A_EXP,
                                scalar2=B_EXP, op0=OP.mult, op1=OP.add)
        xq_sb = singles.tile([P, NR, C], F32)
        nc.gpsimd.dma_start(out=xq_sb, in_=xqf.rearrange("(n p) d -> p n d", p=P))
        grow_sb = singles.tile([P, 2 * C], F32)
        grow_ap = grow[:]
        grow_bc = bass.AP(tensor=grow_ap.tensor, offset=grow_ap.offset,
                          ap=[[0, P]] + list(grow_ap.ap))
        nc.sync.dma_start(out=grow_sb, in_=grow_bc)
        gamma_sb = grow_sb[:, 0:C]
        beta_sb = grow_sb[:, C:2 * C]

        # ---- persistent buffers ----
        ot8 = singles.tile([P, ND, H, RQ], FP8)      # all-heads attn-out (x16)
        ln_out = singles.tile([P, NR, C], F32)
        out_r = out.rearrange("(n p) d -> p n d", p=P)

        # ---- PE warmup: ramp the clock while input DMAs land ----
        wps = pp.tile([P, 2 * QH], F32, tag="pp", name="wps")
        for _ in range(24):
            nc.tensor.matmul(wps[:, 0:P], lhsT=ones8[:], rhs=ones8[:],
                             start=True, stop=True, perf_mode=DR)

        # w projection part for one head -> w8 [p(d_lo), d2, t] fp8 (x1/16);
        # part idx in 0..3 covers (d2, t2) - emitted spread across pass 0
        def emit_wproj_part(h, w8, idx):
            d2, t2 = idx // 2, idx % 2
            ps = pp.tile([P, 2 * QH], F32, tag="pp")
            for j in range(2):
                nc.tensor.matmul(
                    ps[:, j * QH:(j + 1) * QH],
                    lhsT=m_sb[:, :, h, d2 * P:(d2 + 1) * P],
                    rhs=xt_sb[:, :, (2 * t2 + j) * QH:(2 * t2 + j + 1) * QH],
                    start=True, stop=True, perf_mode=DR,
                )
            nc.vector.tensor_scalar_mul(
                out=w8[:, d2, 2 * t2 * QH:(2 * t2 + 2) * QH], in0=ps,
                scalar1=1.0 / 16.0)

        w8 = wpool.tile([P, ND, S], FP8, tag="w")
        for idx in range(4):
            emit_wproj_part(0, w8, idx)

        # fc (PSUM-accumulated over all heads) + residual + LN for one
        # 128-row tile; emitted inside the last head so it overlaps
        def emit_fc_ln(rt):
            psf = pp.tile([P, 2 * QH], F32, tag="pp", name="psf")
            for h2 in range(H):
                nc.tensor.matmul(
                    psf[:, 0:C],
                    lhsT=ot8[:, :, h2, rt * P:(rt + 1) * P],
                    rhs=g_sb[:, :, h2, :],
                    start=(h2 == 0), stop=(h2 == H - 1), perf_mode=DR,
                )
            t = lnpool.tile([P, C], F32, tag="lnt")
            nc.vector.scalar_tensor_tensor(
                out=t, in0=psf[:, 0:C], scalar=1.0 / 512.0, in1=xq_sb[:, rt],
                op0=OP.mult, op1=OP.add)
            stats = lnpool.tile([P, 6], F32, tag="stats")
            nc.vector.bn_stats(out=stats, in_=t)
            mv = lnpool.tile([P, 2], F32, tag="mv")
            nc.vector.bn_aggr(out=mv, in_=stats)
            sd = lnpool.tile([P, 1], F32, tag="sd")
            nc.scalar.activation(out=sd, in_=mv[:, 1:2], func=AF.Sqrt,
                                 bias=eps_t, scale=1.0)
            rstd = lnpool.tile([P, 1], F32, tag="rstd")
            nc.vector.reciprocal(out=rstd, in_=sd)
            if plain_affine:  # gamma == 1, beta == 0
                nc.vector.tensor_scalar(
                    out=ln_out[:, rt], in0=t, scalar1=mv[:, 0:1],
                    scalar2=rstd, op0=OP.subtract, op1=OP.mult)
            else:
                nc.vector.tensor_scalar(out=t, in0=t, scalar1=mv[:, 0:1],
                                        scalar2=rstd, op0=OP.subtract,
                                        op1=OP.mult)
                nc.vector.tensor_tensor(out=t, in0=t, in1=gamma_sb, op=OP.mult)
                nc.vector.tensor_tensor(out=ln_out[:, rt], in0=t, in1=beta_sb,
                                        op=OP.add)
            nc.sync.dma_start(out=out_r[:, rt:rt + 1, :],
                              in_=ln_out[:, rt:rt + 1])

        # ---- head loop ----
        for h in range(H):
            e8 = epool.tile([P, NT, RQ], FP8, tag="e")
            rs_ps = prs.tile([P, RQ], F32, tag="rs")
            st_tiles = {}

            def emit_score(t):
                st = pp.tile([P, RQ], F32, tag="pp", name="st")
                for j in range(2):
                    nc.tensor.matmul(
                        st[:, j * QH:(j + 1) * QH],
                        lhsT=w8[:, :, t * P:(t + 1) * P],
                        rhs=xt_sb[:, :, j * QH:(j + 1) * QH],
                        start=True, stop=True, perf_mode=DR,
                    )
                st_tiles[t] = st

            def emit_exp(t):
                st = st_tiles.pop(t)
                if t in TRICK_T:
                    ti = tpool.tile([P, RQ], I32, tag="ti")
                    nc.vector.tensor_scalar(
                        out=ti, in0=st, scalar1=float(SCALE * A_EXP),
                        scalar2=ebt_sb[:, h, t:t + 1],
                        op0=OP.mult, op1=OP.add)
                    nc.vector.tensor_copy(out=e8[:, t], in_=ti[:].bitcast(F32))
                else:
                    nc.scalar.activation(out=e8[:, t], in_=st, func=AF.Exp,
                                         bias=eba_sb[:, h, t:t + 1],
                                         scale=float(SCALE))

            # pass 0: scores (pipelined two ahead) + exp + rowsum + attnV qh0
            ot_a = [pot.tile([P, QH], F32, tag="ot", name=f"ota{d2}")
                    for d2 in range(ND)]
            emit_score(0)
            emit_exp(0)
            emit_score(1)
            emit_exp(1)
            w8_next = (wpool.tile([P, ND, S], FP8, tag="w", name="w8n")
                       if h + 1 < H else None)
            for tp in range(NP2):
                if tp + 1 < NP2:
                    emit_score(2 * tp + 2)
                    emit_exp(2 * tp + 2)
                    emit_score(2 * tp + 3)
                    emit_exp(2 * tp + 3)
                pair = slice(2 * tp, 2 * tp + 2)
                for j in range(2):
                    nc.tensor.matmul(rs_ps[:, j * QH:(j + 1) * QH],
                                     lhsT=ones8[:],
                                     rhs=e8[:, pair, j * QH:(j + 1) * QH],
                                     start=(tp == 0), stop=(tp == NP2 - 1),
                                     perf_mode=DR)
                for d2 in range(ND):
                    nc.tensor.matmul(
                        ot_a[d2],
                        lhsT=xr8_sb[:, pair, d2 * P:(d2 + 1) * P],
                        rhs=e8[:, pair, 0:QH],
                        start=(tp == 0), stop=(tp == NP2 - 1),
                        perf_mode=DR,
                    )
                # next head's w projection rides along, one part per pair
                if w8_next is not None and 2 <= tp <= 5:
                    emit_wproj_part(h + 1, w8_next, tp - 2)

            rcp_f = rpool.tile([P, RQ], F32, tag="rcp")
            nc.vector.reciprocal_approx_fast(out=rcp_f, in_=rs_ps)
            for d2 in range(ND):
                nc.vector.tensor_tensor(
                    out=ot8[:, d2, h, 0:QH], in0=ot_a[d2],
                    in1=rcp_f[:, 0:QH], op=OP.mult)

            # on the last head, fc+LN of the qh0 row tiles overlaps qh1
            if h == H - 1:
                for rt in range(NR // 2):
                    emit_fc_ln(rt)

            # pass 1: attnV qh1
            ot_b = [pot.tile([P, QH], F32, tag="ot", name=f"otb{d2}")
                    for d2 in range(ND)]
            for tp in range(NP2):
                pair = slice(2 * tp, 2 * tp + 2)
                for d2 in range(ND):
                    nc.tensor.matmul(
                        ot_b[d2],
                        lhsT=xr8_sb[:, pair, d2 * P:(d2 + 1) * P],
                        rhs=e8[:, pair, QH:RQ],
                        start=(tp == 0), stop=(tp == NP2 - 1),
                        perf_mode=DR,
                    )
            if h + 1 < H:
                for d2 in range(ND):
                    nc.vector.tensor_tensor(
                        out=ot8[:, d2, h, QH:RQ], in0=ot_b[d2],
                        in1=rcp_f[:, QH:RQ], op=OP.mult)
                w8 = w8_next
            else:
                # last head: normalize + fc + LN per row tile for a tight tail
                for rt in range(NR // 2, NR):
                    qsl = slice(rt * P, (rt + 1) * P)
                    bsl = slice((rt - NR // 2) * P, (rt - NR // 2 + 1) * P)
                    for d2 in range(ND):
                        nc.vector.tensor_tensor(
                            out=ot8[:, d2, h, qsl], in0=ot_b[d2][:, bsl],
                            in1=rcp_f[:, qsl], op=OP.mult)
                    emit_fc_ln(rt)

    nc.finalize()
    return nc


_NC = {}


def _get_nc(plain_affine: bool = False):
    # score bias rides the eba input tensor (no graph change); the trivial
    # gamma/beta fast path is a build-time flag
    if plain_affine not in _NC:
        _NC[plain_affine] = build_nc(plain_affine)
    return _NC[plain_affine]


def _to_fp8(a):
    import ml_dtypes
    return np.clip(np.asarray(a, np.float32), -240.0, 240.0).astype(
        ml_dtypes.float8_e4m3)


def _wproj8(w, scale):  # [H, C, C] -> [P, ND, H, C] fp8
    w = np.asarray(w, np.float32) * scale
    return np.ascontiguousarray(_to_fp8(
        w.reshape(H, ND, P, C).transpose(2, 1, 0, 3)))


def make_in_maps(inputs):
    x = np.asarray(inputs["x"], dtype=np.float32)
    Wq = np.asarray(inputs["Wq"], np.float32)
    Wk = np.asarray(inputs["Wk"], np.float32)
    Wv = np.asarray(inputs["Wv"], np.float32)
    wfc_f = np.asarray(inputs["Wfc"], np.float32)
    bq = np.asarray(inputs["bq"], np.float32)
    bk = np.asarray(inputs["bk"], np.float32)
    bfc_eff = (np.asarray(inputs["bfc"], np.float32).ravel()
               + np.asarray(inputs["bv"], np.float32).ravel() @ wfc_f)

    # scores = x @ M @ x^T with M = Wq @ Wk^T ; lhsT block needs M^T = Wk@Wq^T
    mT = np.einsum("hcx,hdx->hcd", Wk, Wq)           # [H, C, C] = (WqWk^T)^T
    gfc = np.einsum("hcd,hde->hce", Wv, wfc_f.reshape(H, C, C))
    wkbq = np.einsum("hcd,hd->hc", Wk, bq)           # [H, C]
    biased = bool(np.abs(bq).max() > 0)
    gamma = np.asarray(inputs["gamma"], np.float32).ravel()
    beta = np.asarray(inputs["beta"], np.float32).ravel()
    plain_affine = bool((gamma == 1.0).all() and (beta == 0.0).all())

    shared = {
        "m8": _wproj8(mT, 16.0),
        "g8": _wproj8(gfc, 32.0),
        "grow": np.ascontiguousarray(np.concatenate([
            np.asarray(inputs["gamma"], np.float32).ravel(),
            np.asarray(inputs["beta"], np.float32).ravel(),
        ])),
    }
    x8 = _to_fp8(x)
    in_maps = []
    for core in range(8):
        b, r0 = core // 2, (core % 2) * RQ
        m = dict(shared)
        xroll = np.roll(x8[b], -r0, axis=0)          # own query rows first
        m["xt8"] = np.ascontiguousarray(
            xroll.T.reshape(ND, P, S).transpose(1, 0, 2))
        m["x8r"] = np.ascontiguousarray(
            xroll.reshape(NT, P, C).transpose(1, 0, 2))
        m["xqf"] = np.ascontiguousarray(x[b, r0:r0 + RQ] + bfc_eff[None, :])
        if biased:
            xrf = np.roll(x[b], -r0, axis=0)
            beta_sc = xrf @ wkbq.T + (bq * bk).sum(-1)[None, :]   # [S, H]
            ebv = (EXP_BIAS + SCALE * beta_sc).T.reshape(H, NT, P)
            m["eba"] = np.ascontiguousarray(
                ebv.transpose(2, 0, 1).astype(np.float32))
        else:
            m["eba"] = np.full((P, H, NT), EXP_BIAS, np.float32)
        in_maps.append(m)
    return in_maps, plain_affine


def assemble(results):
    out = np.empty((B, S, C), dtype=np.float32)
    for core in range(8):
        b, r0 = core // 2, (core % 2) * RQ
        out[b, r0:r0 + RQ] = results[core]["out"]
    return out


def kernel(**inputs) -> np.ndarray:
    from concourse.bass_utils import run_bass_kernel_spmd

    in_maps, plain_affine = make_in_maps(inputs)
    nc = _get_nc(plain_affine)
    res = run_bass_kernel_spmd(nc, in_maps, core_ids=list(range(8)))
    return assemble(res.results)
